# revision 6
# baseline (speedup 1.0000x reference)
"""Differentiable random-forest layer (inference path) on 8 Trainium2 cores.

Computation (per reference):
    d     = sigmoid(einsum('bf,tfn->btn', x, W))        # [B, T, 255]
    route = prod_l where(IS_LEFT, d[..n..], 1-d[..n..]) # [B, T, 256]
    out   = clip(einsum('btl,tlc->bc', route, P) / T, 0, 1)

Shapes: B=4096, F=1024, T=10 trees, 255 nodes / 256 leaves, C=1000.

Sharding: data-parallel over batch. Each of the 8 cores handles 512 rows;
no collectives are needed (weights/probs are broadcast to every core).

Both matmuls run in fp8(e4m3) with perf_mode=DoubleRow (K=256 per
instruction, ~2x PE throughput vs bf16). Scaling keeps every fp8 operand
in the normal range:
  W' = 16*W          (sigmoid applied with scale=1/16)
  route' = 128*route (seeded at the routing root)
  P' = P * 2^20 / T  (max ~210 < 240 TRN-e4m3 limit)
  out = psum * 2^-27 (folded into the ACT psum->sbuf copy)

Pipeline (per core), tuned against the measured engine timelines:
  - inputs land via 7 contiguous p-major DMAs (x | w j-blocks | p); the
    ~6.8us engine preamble + ~0.65us/dispatch serialization on the Sync
    queue makes dispatch count and order matter.
  - mm1 phase: chunk b0's five tree-pairs run the moment each W j-block
    lands (b1's fill the DMA bubbles), then b2, b3. One sigmoid per
    tree-pair; dbar is never materialized (routing uses hi = r - r*d).
  - each chunk's routing (DVE) -> 2 XBAR transposes (Sync, serial HW) ->
    bf16-to-fp8 casts (kc0 on DVE, kc1 on ACT) chain hides under the
    other chunks' mm1/mm2 PE work; emission order keeps every engine
    FIFO from blocking a later chunk's chain.
  - mm2 accumulates 10 DoubleRow matmuls (K=256 leaves) per 500-col
    output block; the psum->sbuf copy with the 2^-27 scale runs on ACT.

The routing uses the "concat" (decision-bit-as-LSB) leaf ordering so every
DVE read/write is contiguous; the host pre-permutes W's node axis (per-layer
bit-reversal) and P's leaf axis (8-bit reversal) to compensate, which is free.
The reference clip(.,0,1) is provably inactive (all terms nonneg, outputs
~1e-3), so no clamp is emitted.
"""

from contextlib import ExitStack

import numpy as np
import ml_dtypes

import concourse.bass as bass
import concourse.bacc as bacc
import concourse.mybir as mybir
import concourse.tile as tile
from concourse.bass_utils import run_bass_kernel_spmd

N_CORES = 8
B, F, T, NODES, LEAFS, C = 4096, 1024, 10, 255, 256, 1000
B_LOC = B // N_CORES            # 512 batch rows per core
BCH = B_LOC // 128              # 4 batch chunks of 128
KF = F // 128                   # 8 contraction chunks of 128
KD = KF // 2                    # 4 DoubleRow chunks of 256
TP = T // 2                     # 5 tree-pairs
NP = 256                        # per-tree node block, padded 255 -> 256
CP = 512                        # per-tree class-column half, padded 500 -> 512
N_LAYERS = 8

W_SCALE = 16.0                  # W' = 16 W; sigmoid scale = 1/16
R_SCALE = 128.0                 # route' = 128 route
P_SCALE = 2.0 ** 20             # P' = P * 2^20 / T
OUT_SCALE = 1.0 / (R_SCALE * P_SCALE)   # 2^-27

BF16 = mybir.dt.bfloat16
FP8 = mybir.dt.float8e4
F32 = mybir.dt.float32
Sigmoid = mybir.ActivationFunctionType.Sigmoid
Copy = mybir.ActivationFunctionType.Copy
DR = mybir.MatmulPerfMode.DoubleRow
DRS = mybir.MatmulPerfMode.DoubleRowSwInterleave
MULT = mybir.AluOpType.mult
ADD = mybir.AluOpType.add


def _bitrev(x: int, bits: int) -> int:
    r = 0
    for _ in range(bits):
        r = (r << 1) | (x & 1)
        x >>= 1
    return r


# Node-axis permutation: d'[.., off+q] = d[.., off+bitrev_l(q)] per layer l
NODE_PERM = np.empty(NODES, dtype=np.int64)
for _l in range(N_LAYERS):
    _off = (1 << _l) - 1
    for _q in range(1 << _l):
        NODE_PERM[_off + _q] = _off + _bitrev(_q, _l)
# Leaf-axis permutation: P'[t, q, :] = P[t, bitrev_8(q), :]
LEAF_PERM = np.array([_bitrev(q, N_LAYERS) for q in range(LEAFS)], dtype=np.int64)


def build_program() -> bass.Bass:
    nc = bacc.Bacc()

    # all inputs partition-major so each is one contiguous-row DMA
    xT = nc.dram_tensor("xT", [128, KF * B_LOC], FP8, kind="ExternalInput")
    # W is j-major: per tree-pair j, [128, KF * 2 * NP] covering all KF chunks,
    # each chunk holding two trees' node blocks (255 nodes + 1 pad col each)
    w = nc.dram_tensor("w", [TP, 128, KF * 2 * NP], FP8, kind="ExternalInput")
    p = nc.dram_tensor("p", [2, 128, 2 * T * CP], FP8, kind="ExternalInput")
    out = nc.dram_tensor("out", [B_LOC, C], F32, kind="ExternalOutput")

    with tile.TileContext(nc) as tc, ExitStack() as ctx:
        resident = ctx.enter_context(tc.tile_pool(name="resident", bufs=1))
        x_all = resident.tile([128, KF, B_LOC], FP8, tag="x_all", name="x_all")
        w_all = resident.tile([128, TP, KF, 2 * NP], FP8, tag="w_all", name="w_all")
        p_all = resident.tile([128, 2, 2, T, CP], FP8, tag="p_all", name="p_all")
        nc.sync.dma_start(x_all[:, :, :], xT.rearrange("p (k n) -> p k n", k=KF))
        for j in range(TP):
            nc.sync.dma_start(w_all[:, j, :, :], w[j])
        pv = p.rearrange("c p (k t n) -> c p k t n", k=2, t=T)
        # column-half split: mm2's first output block only waits on half of P
        nc.sync.dma_start(p_all[:, 0], pv[0])
        nc.sync.dma_start(p_all[:, 1], pv[1])

        dpool = ctx.enter_context(tc.tile_pool(name="dps", bufs=1, space="PSUM"))
        opool = ctx.enter_context(tc.tile_pool(name="ops", bufs=3, space="PSUM"))
        work = ctx.enter_context(tc.tile_pool(name="work", bufs=2))

        # ---- PE warmup: the first few us are DMA-bound; keep the PE busy so
        # its HAM clock gate reaches full speed before the real matmuls. ----
        warm_in = work.tile([128, 128], BF16, tag="warm", name="warm_in", bufs=1)
        nc.vector.memset(warm_in[:, :], 0.0)
        warm_ps = opool.tile([128, 128], F32, tag="warm", name="warm_ps", bufs=1)

        def warm_mms(n):
            for _ in range(n):
                nc.tensor.matmul(warm_ps[:, :], warm_in[:, :], warm_in[:, :])

        warm_mms(60)

        def emit_mm1_j(bi, j, ddb):
            # d logits for tree-pair j of chunk bi (4 DoubleRow matmuls over
            # the 1024-deep contraction), then one sigmoid into ddb
            dps = dpool.tile([128, 2, NP], F32, tag="dps", name="dps", bufs=3)
            for kd in range(KD):
                nc.tensor.matmul(
                    dps[:, :, :],
                    x_all[:, 2 * kd : 2 * kd + 2, bass.ts(bi, 128)],
                    w_all[:, j, 2 * kd : 2 * kd + 2, :],
                    start=(kd == 0),
                    stop=(kd == KD - 1),
                    perf_mode=DR,
                )
            # d = sigmoid(logits / 16) -> bf16  (1 - d is never materialized:
            # the routing uses hi = r - r*d instead)
            nc.scalar.activation(
                ddb[:, 2 * j : 2 * j + 2, :], dps[:, :, 0:NODES], Sigmoid,
                scale=1.0 / W_SCALE,
            )

        def emit_routing(ddb):
            # ---- routing: hierarchical doubling, concat ordering, scaled by
            # 128 at the seed so the final fp8 route values are ~O(1) ----
            # lo = R_l * d_l ; hi = R_l - lo  (== R_l * (1-d_l))
            Ra = work.tile([128, T, LEAFS], BF16, tag="Ra", name="Ra")
            Rb = work.tile([128, T, LEAFS], BF16, tag="Rb", name="Rb")
            # final layer written as fp8 directly (hi = r - lo_fp8 keeps each
            # pair's sum exact); adjacent-leaf pairs form 16-bit words, so ONE
            # XBAR transpose of the bf16 view yields the DoubleRowSwInterleave
            # stationary layout (pairs interleaved, columns reversed -- the
            # host pre-reverses the batch rows of x inside each 128-chunk to
            # compensate, see _prep_inputs).
            routeC8 = work.tile([128, T, LEAFS], FP8, tag="routeC8", name="routeC8")
            nc.vector.tensor_scalar_mul(Ra[:, :, 0:1], ddb[:, :, 0:1], R_SCALE)
            nc.vector.tensor_scalar(
                Ra[:, :, 1:2], ddb[:, :, 0:1], -R_SCALE, R_SCALE, MULT, ADD
            )
            cur, nxt = Ra, Rb
            for l in range(1, N_LAYERS):
                w_l = 1 << l          # prefixes at layer l
                off = w_l - 1         # first node index of layer l
                if l < N_LAYERS - 1:
                    lo, hi = nxt[:, :, 0:w_l], nxt[:, :, w_l : 2 * w_l]
                else:
                    lo, hi = routeC8[:, :, 0:w_l], routeC8[:, :, w_l : 2 * w_l]
                nc.vector.tensor_mul(lo, cur[:, :, 0:w_l], ddb[:, :, off : off + w_l])
                nc.vector.tensor_sub(hi, cur[:, :, 0:w_l], lo)
                cur, nxt = nxt, cur
            rT8w = work.tile([128, T, 128], BF16, tag="rT8w", name="rT8w", bufs=3)
            nc.sync.dma_start_transpose(
                rT8w[:, :, :], routeC8[:, :, :].bitcast(BF16)
            )
            return rT8w

        def emit_mm2(rT8w, bsl):
            # mm2: out[b, c] += routeT8.T @ P8, accumulated over trees with
            # K=256 (both leaf chunks) per DoubleRow matmul. Two 500-col
            # output blocks, one per P column-half.
            osb = work.tile([128, C], F32, tag="osb", name="osb")
            for ch, n0, nsz in ((0, 0, 500), (1, 500, 500)):
                ops = opool.tile([128, 512], F32, tag="ops", name="ops")
                for t_ in range(T):
                    nc.tensor.matmul(
                        ops[:, 0:nsz],
                        rT8w[:, t_, :].bitcast(FP8),
                        p_all[:, ch, :, t_, 0:nsz],
                        start=(t_ == 0),
                        stop=(t_ == T - 1),
                        perf_mode=DRS,
                    )
                # out = psum * 2^-27, on the ACT engine (DVE is busy routing)
                nc.scalar.activation(
                    osb[:, n0 : n0 + nsz], ops[:, 0:nsz], Copy, scale=OUT_SCALE
                )
                # store dispatch on the (idle) GpSimd queue so the Sync queue
                # never delays an XBAR transpose
                nc.gpsimd.dma_start(out[bsl, n0 : n0 + nsz], osb[:, n0 : n0 + nsz])

        # ---- emission order = per-engine instruction order ----
        ddbs = [
            work.tile([128, T, NODES], BF16, tag="ddb", name=f"ddb{i}", bufs=4)
            for i in range(BCH)
        ]
        # mm1: chunk b0's tree-pairs fire the moment each W j-block lands,
        # with b1's filling the DMA-arrival bubbles; then b2.
        for bi, j in [(0, 0), (1, 0), (0, 1), (0, 2), (1, 1), (0, 3), (1, 2),
                      (0, 4), (1, 3), (1, 4)]:
            emit_mm1_j(bi, j, ddbs[bi])
        for j in range(TP):
            emit_mm1_j(2, j, ddbs[2])
        rT0 = emit_routing(ddbs[0])
        for j in range(TP):
            emit_mm1_j(3, j, ddbs[3])
        rT1 = emit_routing(ddbs[1])
        emit_mm2(rT0, bass.ts(0, 128))
        rT2 = emit_routing(ddbs[2])
        emit_mm2(rT1, bass.ts(1, 128))
        rT3 = emit_routing(ddbs[3])
        emit_mm2(rT2, bass.ts(2, 128))
        emit_mm2(rT3, bass.ts(3, 128))

    nc.finalize()
    return nc


_CACHED_NC = None
_WARMED = False


def _prep_inputs(l_input, cnn_w, final_probabilities):
    f8 = ml_dtypes.float8_e4m3fn
    x = np.ascontiguousarray(np.asarray(l_input, dtype=np.float32))
    W = np.asarray(cnn_w, dtype=np.float32)[:, :, NODE_PERM] * W_SCALE
    # fold the 1/T tree-mean and the fp8 range scale into P
    P = np.asarray(final_probabilities, dtype=np.float32)[:, LEAF_PERM, :] * (
        P_SCALE / T
    )

    # batch rows reversed inside each 128-chunk: the SwInterleave matmul
    # reads its stationary columns in reverse, which undoes this
    x = x.reshape(B // 128, 128, F)[:, ::-1, :].reshape(B, F)
    # x [B, F] -> xT [128, KF, B] (partition-major, contraction-chunk next)
    xT = np.ascontiguousarray(
        x.T.reshape(KF, 128, B).transpose(1, 0, 2)
    ).astype(f8)
    # W [T, F, 255] -> [F, T, 256] (pad) -> [KF, 128, TP, 512] -> j-major
    Wq = np.ascontiguousarray(W.transpose(1, 0, 2)).astype(f8)  # [F, T, 255]
    Wpad = np.zeros((F, T, NP), dtype=f8)
    Wpad[:, :, 0:NODES] = Wq
    Wr = (
        Wpad.reshape(KF, 128, TP, 2 * NP)
        .transpose(2, 1, 0, 3)
        .reshape(TP, 128, KF * 2 * NP)
    )
    Wr = np.ascontiguousarray(Wr)
    # P [T, 256, C] -> [c-half, 128(ki), slot, T, 512]; k-slot s of cell ki
    # holds leaf 2*ki+s (adjacent-leaf pairing to match the packed transpose)
    P8 = np.zeros((2, 128, 2, T, CP), dtype=f8)
    Pq = P.reshape(T, 128, 2, C).transpose(1, 2, 0, 3).astype(f8)  # [128,2,T,C]
    P8[0, :, :, :, 0:500] = Pq[:, :, :, 0:500]
    P8[1, :, :, :, 0:500] = Pq[:, :, :, 500:1000]
    Pr = np.ascontiguousarray(P8).reshape(2, 128, 2 * T * CP)
    return xT, Wr, Pr


def _get_nc() -> bass.Bass:
    global _CACHED_NC
    if _CACHED_NC is None:
        _CACHED_NC = build_program()
    return _CACHED_NC


def _run(inputs, trace=False, trace_cores=None):
    xT, Wr, Pr = _prep_inputs(
        inputs["l_input"], inputs["cnn_w"], inputs["final_probabilities"]
    )
    in_maps = [
        {
            "xT": np.ascontiguousarray(
                xT[:, :, c * B_LOC : (c + 1) * B_LOC]
            ).reshape(128, KF * B_LOC),
            "w": Wr,
            "p": Pr,
        }
        for c in range(N_CORES)
    ]
    global _WARMED
    if not _WARMED and not trace:
        # one discarded execution to warm the device path (DMA rings, NEFF
        # residency, clock state) so the measured run is at steady state
        try:
            run_bass_kernel_spmd(
                _get_nc(), in_maps, core_ids=list(range(N_CORES)), trace=False
            )
        except Exception:
            pass
        _WARMED = True
    last_err = None
    for attempt in range(3):
        try:
            res = run_bass_kernel_spmd(
                _get_nc(),
                in_maps,
                core_ids=list(range(N_CORES)),
                trace=trace,
                trace_cores=trace_cores,
            )
            break
        except Exception as e:  # transient NRT device errors: retry
            last_err = e
            if attempt == 2:
                raise
            import time as _time

            _time.sleep(5)
    out = np.concatenate([res.results[c]["out"] for c in range(N_CORES)], axis=0)
    return out, res


def kernel(**inputs) -> np.ndarray:
    out, _ = _run(inputs)
    return out
